# revision 1
# baseline (speedup 1.0000x reference)
"""Trainium2 Bass kernel for CircleProjectionLayer (ball projection, r=1).

out = center + d * min(1, 1/||d||),  d = x - center,  shapes [8388608, 3] f32.

Sharding: pure data parallel — batch split 8 ways, one shard per NeuronCore.
Per-core layout: the [1048576, 3] shard is viewed flat as [128, 24576] so each
SBUF partition holds 8192 complete (x,y,z) rows contiguously; chunks of
W floats per partition stream through SBUF.

Engine split (all fp32):
  DVE   : d = x-c; s01 = dx^2+dy^2; s = max(s01,eps)+dz^2 (fused
          scalar_tensor_tensor); m_k = d_k * scale (x3, into the sq tile)
  ACT   : sq = Square(d); l = Ln(s); rl = Relu(l); scale = Exp(-0.5*rl)
          (one pre-placed table load: natural_log_exp_and_others covers all 4;
           exp(-0.5*relu(ln(s))) == min(1, rsqrt(s)) with an exact clamp at 1)
  GPSIMD: out = m + c  (into the d tile; DVE ops are all 1-port TT class, so
          no shared-SBUF-port contention)
  DMA   : x-in + center-in on the SP HWDGE ring; out on the Pool/SWDGE ring so
          late output DMAs can't head-of-line-block the input stream.
"""

import sys

sys.path.insert(0, "/opt/trn_rl_repo")

from contextlib import ExitStack

import numpy as np

import concourse.bass as bass
import concourse.tile as tile
from concourse import bacc, mybir
from concourse.bass_utils import run_bass_kernel_spmd
from concourse.hw_specs import get_activation_tables

F32 = mybir.dt.float32
AF = mybir.ActivationFunctionType
ALU = mybir.AluOpType

B = 8388608
N_CORES = 8
B_CORE = B // N_CORES          # 1048576 rows per core
P = 128
FPP = B_CORE * 3 // P          # 24576 floats per partition

_EPS = 1e-30
_ACT_SET = "natural_log_exp_and_others"


def _preload_act_table(nc):
    """Pre-place one LoadActFuncSet for the set containing Square/Ln/Relu/Exp
    so Bacc.insert_act_table_loads doesn't thrash between greedy choices."""
    tables = list(get_activation_tables(nc.m.arch).keys())
    set_id = tables.index(_ACT_SET)
    inst = mybir.InstLoadActFuncSet(
        name=nc.get_next_instruction_name(), act_func_set_id=set_id, ins=[], outs=[]
    )
    return nc.scalar.add_instruction(inst)


def _build(W=3072, bufs=3, c_ring="sp", out_ring="pool", preload_act=True,
           schedule=None, loop_reps=1, sub_split=0.0, bufs_io=None):
    """`schedule`: optional explicit list of chunk widths (floats/partition,
    each a multiple of 3, summing to FPP). Default: uniform chunks of W.
    `loop_reps`: wrap the whole schedule in a hardware For_i loop (used only
    for benchmarking steady-state HW time via wall-clock deltas)."""
    if schedule is None:
        assert W % 3 == 0 and FPP % W == 0
        schedule = [W] * (FPP // W)
    assert sum(schedule) == FPP and all(w % 3 == 0 for w in schedule)
    W = max(schedule)

    nc = bacc.Bacc("TRN2", target_bir_lowering=False, debug=False)

    x = nc.dram_tensor("x", [B_CORE, 3], F32, kind="ExternalInput")
    c = nc.dram_tensor("center", [B_CORE, 3], F32, kind="ExternalInput")
    o = nc.dram_tensor("out", [B_CORE, 3], F32, kind="ExternalOutput")

    xr = x.ap().rearrange("(p f) c -> p (f c)", p=P)
    cr = c.ap().rearrange("(p f) c -> p (f c)", p=P)
    orr = o.ap().rearrange("(p f) c -> p (f c)", p=P)

    rings = {"sp": nc.sync, "act": nc.scalar, "pool": nc.gpsimd}

    with tile.TileContext(nc) as tc, ExitStack() as ctx:
        if preload_act:
            _preload_act_table(nc)

        io_bufs = bufs_io if bufs_io is not None else bufs
        xp = ctx.enter_context(tc.tile_pool(name="xp", bufs=io_bufs))
        cp = ctx.enter_context(tc.tile_pool(name="cp", bufs=io_bufs))
        dp = ctx.enter_context(tc.tile_pool(name="dp", bufs=bufs))
        sqp = ctx.enter_context(tc.tile_pool(name="sqp", bufs=bufs))
        sp = ctx.enter_context(tc.tile_pool(name="sp", bufs=bufs))

        c_dma = rings[c_ring]
        o_dma = rings[out_ring]

        import contextlib
        loop_cm = tc.For_i(0, loop_reps, 1) if loop_reps > 1 else contextlib.nullcontext()
        with loop_cm:
            emit_body(nc, tc, schedule, W, bufs, c_dma, o_dma,
                      xp, cp, dp, sqp, sp, xr, cr, orr, sub_split=sub_split)

    nc.compile()
    return nc


def emit_body(nc, tc, schedule, W, bufs, c_dma, o_dma, xp, cp, dp, sqp, sp,
              xr, cr, orr, sub_split=0.0):
        off = 0
        for i, w in enumerate(schedule):
            r = w // 3
            xt = xp.tile([P, W], F32, name="xt", tag="xt")[:, :w]
            nc.sync.dma_start(xt[:, :], xr[:, off : off + w])
            ct = cp.tile([P, W], F32, name="ct", tag="ct")[:, :w]
            c_dma.dma_start(ct[:, :], cr[:, off : off + w])

            dt = dp.tile([P, W], F32, name="dt", tag="dt")[:, :w]
            if sub_split > 0.0:
                # Rebalance: tail fraction of the subtract runs on GPSIMD
                # (HW is DVE-bound; Pool has slack).
                w2 = int(w * (1.0 - sub_split)) // 96 * 96
                nc.vector.tensor_sub(dt[:, :w2], xt[:, :w2], ct[:, :w2])
                nc.gpsimd.tensor_sub(dt[:, w2:], xt[:, w2:], ct[:, w2:])
            else:
                nc.vector.tensor_sub(dt[:, :], xt[:, :], ct[:, :])

            sq = sqp.tile([P, W], F32, name="sq", tag="sq")[:, :w]
            nc.scalar.activation(sq[:, :], dt[:, :], AF.Square)

            # Small per-row chain ping-pongs between two [P, R] tiles (A, B).
            sq3 = sq.rearrange("p (r c) -> p r c", c=3)
            ta = sp.tile([P, W // 3], F32, name="ta", tag="ta")[:, :r]
            nc.vector.tensor_add(ta[:, :], sq3[:, :, 0], sq3[:, :, 1])
            tb = sp.tile([P, W // 3], F32, name="tb", tag="tb")[:, :r]
            nc.vector.scalar_tensor_tensor(
                tb[:, :], ta[:, :], _EPS, sq3[:, :, 2], ALU.max, ALU.add
            )
            nc.scalar.activation(ta[:, :], tb[:, :], AF.Ln)
            nc.scalar.activation(tb[:, :], ta[:, :], AF.Relu)
            sc = sp.tile([P, W // 3], F32, name="sc", tag="sc")[:, :r]
            nc.scalar.activation(sc[:, :], tb[:, :], AF.Exp, scale=-0.5)

            # m_k = d_k * scale, into the sq tile (squares are dead now).
            m3 = sq.rearrange("p (r c) -> p r c", c=3)
            d3 = dt.rearrange("p (r c) -> p r c", c=3)
            for k in range(3):
                nc.vector.tensor_mul(m3[:, :, k], d3[:, :, k], sc[:, :])

            # out = m + c, into the d tile (d is dead now).
            nc.gpsimd.tensor_add(dt[:, :], sq[:, :], ct[:, :])

            o_dma.dma_start(orr[:, off : off + w], dt[:, :])
            off += w


_NC = None

# Uniform 3072-wide chunks with a 2x1536 taper at the end: the taper shortens
# the pipeline tail (last chunk's compute chain + out-DMA trail the stream).
_SCHEDULE = [3072] * 7 + [1536] * 2


def _get_nc():
    global _NC
    if _NC is None:
        _NC = _build(schedule=_SCHEDULE)
    return _NC


def kernel(**inputs):
    x = np.asarray(inputs["x"], dtype=np.float32)
    center = np.asarray(inputs["center"], dtype=np.float32)
    assert x.shape == (B, 3) and center.shape == (B, 3)

    xs = x.reshape(N_CORES, B_CORE, 3)
    cs = center.reshape(N_CORES, B_CORE, 3)
    in_maps = [
        {"x": np.ascontiguousarray(xs[i]), "center": np.ascontiguousarray(cs[i])}
        for i in range(N_CORES)
    ]

    nc = _get_nc()
    res = run_bass_kernel_spmd(nc, in_maps, list(range(N_CORES)))
    out = np.concatenate([res.results[i]["out"] for i in range(N_CORES)], axis=0)
    return out.astype(np.float32, copy=False)


if __name__ == "__main__":
    nc = _get_nc()
    print("build ok")

